# revision 10
# baseline (speedup 1.0000x reference)
"""Trainium2 Bass kernel for nn_CAFVBlock (audio/video cross-attention fusion).

v2 strategy (8 NeuronCores, SPMD): core = 2*b + h handles sample b and output
channel residues r in {2h, 2h+1} (cv = 4*ca + r).

Post-mu critical path is split across three engines:
  - residue 1 gate: ACT relu (per-channel scale/bias) -> bf16 z, DVE tree-sums
  - residue 0 gate: DVE tensor_scalar relu(x + c) with the sign of the channel
    scale handled algebraically: for s<0, sum_f relu(s*(x+c)) =
    s*(sum_f (x+c) - sum_f max(x+c,0)) which reuses the per-ta sums SA.
    This runs at 2x DVE rate (f32 SBUF) and the bf16 tree adds at 2x.
  - fusion tail: out_i = P_i + gs_i * W_i with W/P precomputed on GpSimd
    before the gate sums land.
DMA: audio first (5 chunks on the sync ring), video+cw on the scalar ring,
single merged output DMA.  GroupNorm stats on-device as in v1 (weighted sums
+ matmul-with-ones partition reduction + Newton rsqrt).
"""
import os
import sys
import numpy as np

for _p in ("/opt/trn_rl_repo",):
    if _p not in sys.path and os.path.isdir(_p):
        sys.path.insert(0, _p)

import concourse.bass as bass
import concourse.tile as tile
from concourse import bacc, mybir
from concourse.bass_utils import run_bass_kernel_spmd

F32 = mybir.dt.float32
BF16 = mybir.dt.bfloat16
I32 = mybir.dt.int32
AF = mybir.ActivationFunctionType
ALU = mybir.AluOpType
RSQRT_MAGIC = 0x5F3759DF

B, Ca, Cv, NH = 4, 128, 512, 8
Ta, F, Tv = 64, 64, 256
REP = Cv // Ca   # 4
EPS = 1e-5
N1 = Cv * Ta * F
N3 = Cv * NH * Tv
N4 = Cv * Tv

# cw column layout (host-precomputed constants, one row per ca partition)
C_W1S, C_W2S, C_W1SQ, C_W2SQ, C_WB1, C_WB2 = 0, 1, 2, 3, 4, 5
C_VT1 = 6    # 16 cols: [V3S(4), V4S(4), VB3(4), VB4(4)]  (T1v-weighted)
C_VT2 = 22   # 8 cols:  [V3SQ(4), V4SQ(4)]                (T2v-weighted)
C_W2G2, C_BG2, C_G2, C_BE2 = 30, 32, 34, 36     # +i for i in {0,1}
C_W1G1, C_BG1, C_G1, C_BE1 = 38, 40, 42, 44
C_W3GM, C_BG3M, C_G3M, C_BE3M = 46, 48, 50, 52
C_W4G4, C_BG4, C_G4, C_BE4 = 54, 56, 58, 60
C_S0, C_M1, C_M2, C_WNI, C_WNIF = 62, 63, 64, 65, 66   # residue-0 DVE gate
NCW = 67

_CACHE = {}
LAST_EXEC_NS = None
LAST_RESULTS = None


def _derive_invs(nc, sp, magic, s_ap, q_ap, qb_ap, imms, tag, mu_ready=False,
                 n_iter=2):
    """inv = 1/sqrt(var+eps) and muinv = mu*inv from weighted sums ([128,2])
    via the int rsqrt bit-trick + Newton iterations on the DVE."""
    v = nc.vector
    invN_a, mua_a, qa_a, invN_b, mua_b, qa_b = imms
    if mu_ready:
        mu_ap = s_ap
    else:
        mu = sp.tile([128, 2], F32, tag=f"mu{tag}")
        v.tensor_scalar(mu[:, 0:1], s_ap[:, 0:1], invN_a, mua_a, ALU.mult, ALU.add)
        v.tensor_scalar(mu[:, 1:2], s_ap[:, 1:2], invN_b, mua_b, ALU.mult, ALU.add)
        mu_ap = mu[:]
    if qb_ap is not None:
        qbs = sp.tile([128, 2], F32, tag=f"qbs{tag}")
        v.tensor_copy(qbs[:], qb_ap)
        qs = sp.tile([128, 2], F32, tag=f"qs{tag}")
        v.tensor_tensor(qs[:], q_ap, qbs[:], ALU.add)
        qs_ap = qs[:]
    else:
        qs_ap = q_ap
    qn = sp.tile([128, 2], F32, tag=f"qn{tag}")
    v.tensor_scalar(qn[:, 0:1], qs_ap[:, 0:1], invN_a, qa_a, ALU.mult, ALU.add)
    v.tensor_scalar(qn[:, 1:2], qs_ap[:, 1:2], invN_b, qa_b, ALU.mult, ALU.add)
    mm = sp.tile([128, 2], F32, tag=f"mm{tag}")
    v.tensor_tensor(mm[:], mu_ap, mu_ap, ALU.mult)
    varp = sp.tile([128, 2], F32, tag=f"varp{tag}")
    v.tensor_tensor(varp[:], qn[:], mm[:], ALU.subtract)
    half = sp.tile([128, 2], I32, tag=f"half{tag}")
    v.tensor_scalar(half[:], varp[:].bitcast(I32), 1, None, ALU.arith_shift_right)
    yi = sp.tile([128, 2], I32, tag=f"yi{tag}")
    v.tensor_tensor(yi[:], magic[:, 0:2], half[:], ALU.subtract)
    xh = sp.tile([128, 2], F32, tag=f"xh{tag}")
    v.tensor_scalar(xh[:], varp[:], 0.5, None, ALU.mult)
    y = yi[:].bitcast(F32)
    for it in range(n_iter):
        t2 = sp.tile([128, 2], F32, tag=f"t2{tag}{it}")
        v.tensor_tensor(t2[:], y, y, ALU.mult)
        v.tensor_tensor(t2[:], t2[:], xh[:], ALU.mult)
        v.tensor_scalar(t2[:], t2[:], -1.0, 1.5, ALU.mult, ALU.add)
        yn = sp.tile([128, 2], F32, tag=f"yn{tag}{it}")
        v.tensor_tensor(yn[:], y, t2[:], ALU.mult)
        y = yn[:]
    inv = y
    muinv = sp.tile([128, 2], F32, tag=f"muinv{tag}")
    v.tensor_tensor(muinv[:], mu_ap, inv, ALU.mult)
    return inv, muinv


def _coef_pair(nc, sp, cw, base, inv_ap, muinv_ap, has_be, tag, v=None):
    """alpha/beta for BOTH i in one [128,2] tile each (cw stores negated
    gamma so only mult/add are needed; Pool rejects subtract)."""
    if v is None:
        v = nc.vector
    invb = inv_ap.broadcast_to((128, 2))
    alpha = sp.tile([128, 2], F32, tag=f"al{tag}")
    v.tensor_tensor(alpha[:], cw[:, base:base + 2], invb, ALU.mult)
    beta = sp.tile([128, 2], F32, tag=f"be{tag}")
    v.tensor_tensor(beta[:], cw[:, base + 2:base + 4], invb, ALU.mult)
    tb = sp.tile([128, 2], F32, tag=f"tb{tag}")
    v.tensor_tensor(tb[:], cw[:, base + 4:base + 6],
                    muinv_ap.broadcast_to((128, 2)), ALU.mult)
    v.tensor_tensor(beta[:], beta[:], tb[:], ALU.add)
    if has_be:
        v.tensor_tensor(beta[:], beta[:], cw[:, base + 6:base + 8], ALU.add)
    return alpha, beta


def build_program(imms, has_be):
    nc = bacc.Bacc("TRN2", target_bir_lowering=False, debug=False, num_devices=8)

    audio_s = nc.dram_tensor("audio_s", [128, Ta * F], F32, kind="ExternalInput")
    video_f = nc.dram_tensor("video_f", [128, REP * Tv], F32, kind="ExternalInput")
    cw_d = nc.dram_tensor("cw", [128, NCW], F32, kind="ExternalInput")
    out_d = nc.dram_tensor("out_c", [128, 2 * Tv], F32, kind="ExternalOutput")

    # audio chunks: three 1024 then two 512 (finer tail for latency)
    offs = [0, 1024, 2048, 3072, 3584]
    sizes = [1024, 1024, 1024, 512, 512]
    NCH = len(offs)
    qb_zero = has_be[4] if len(has_be) > 4 else False
    fast_gate = not has_be[1]
    fast_val = not has_be[0]
    assert fast_gate, "residue-0 DVE gate path assumes p2_b==0, p2_be==0"

    with tile.TileContext(nc) as tc:
        with (
            tc.tile_pool(name="big", bufs=1) as bigp,
            tc.tile_pool(name="z", bufs=2) as zp,
            tc.tile_pool(name="scr", bufs=2) as scrp,
            tc.tile_pool(name="sp", bufs=1) as sp,
            tc.tile_pool(name="psum", bufs=2, space="PSUM") as psp,
        ):
            v = nc.vector
            g = nc.gpsimd
            A = bigp.tile([128, Ta * F], F32, tag="A")
            vf = bigp.tile([128, REP * Tv], F32, tag="vf")
            cw = bigp.tile([128, NCW], F32, tag="cw")
            ones = bigp.tile([128, 128], F32, tag="ones")
            magic = bigp.tile([128, 2], I32, tag="magic")

            # ---- input DMAs: audio chunks first (sync ring), video + cw on
            # the scalar ring so video-path stats can run inside the window.
            VH = REP * Tv // 2
            nc.scalar.dma_start(cw[:], cw_d[:])
            dma_eng = [nc.sync, nc.scalar]
            for c in range(NCH):
                dma_eng[c % 2].dma_start(A[:, offs[c]:offs[c] + sizes[c]],
                                         audio_s[:, offs[c]:offs[c] + sizes[c]])
            nc.scalar.dma_start(vf[:, :VH], video_f[:, :VH])
            nc.sync.dma_start(vf[:, VH:], video_f[:, VH:])
            g.memset(ones[:], 1.0)
            g.memset(magic[:], RSQRT_MAGIC)

            # ---- per-chunk window work: DVE per-ta sums, ACT squares.
            # Video stats are emitted between the early audio chunks so they
            # land inside the DMA window (video arrives on the scalar ring
            # while audio streams on the sync ring).
            SA = sp.tile([128, Ta], F32, tag="SA")
            T2c = sp.tile([128, NCH], F32, tag="T2c")
            sq = scrp.tile([128, 1024], F32, tag="sq")

            def audio_chunk(c):
                v.reduce_sum(SA[:, offs[c] // F:(offs[c] + sizes[c]) // F],
                             A[:, offs[c]:offs[c] + sizes[c]].rearrange(
                                 "p (t f) -> p t f", f=F),
                             axis=mybir.AxisListType.X)
                nc.scalar.activation(sq[:, :sizes[c]],
                                     A[:, offs[c]:offs[c] + sizes[c]], AF.Square,
                                     accum_out=T2c[:, c:c + 1])

            audio_chunk(0)
            audio_chunk(1)

            # ---- video stats (vsq on GpSimd; reduces on DVE)
            T2vc = sp.tile([128, 4], F32, tag="T2vc")
            T1vc = sp.tile([128, 4], F32, tag="T1vc")
            vsq = scrp.tile([128, REP * Tv], F32, tag="vsq")
            for hh in range(2):
                hs = slice(VH * hh, VH * (hh + 1))
                g.tensor_tensor(vsq[:, hs], vf[:, hs], vf[:, hs], ALU.mult)
                v.reduce_sum(T1vc[:, 2 * hh:2 * hh + 2],
                             vf[:, hs].rearrange("p (r t) -> p r t", t=Tv),
                             axis=mybir.AxisListType.X)
                v.reduce_sum(T2vc[:, 2 * hh:2 * hh + 2],
                             vsq[:, hs].rearrange("p (r t) -> p r t", t=Tv),
                             axis=mybir.AxisListType.X)
            pt1 = sp.tile([128, 16], F32, tag="pt1")
            v.tensor_tensor(pt1[:].rearrange("p (g r) -> p g r", r=4),
                            T1vc[:].unsqueeze(1).broadcast_to((128, 4, 4)),
                            cw[:, C_VT1:C_VT1 + 16].rearrange(
                                "p (g r) -> p g r", r=4), ALU.mult)
            pv1 = sp.tile([128, 4], F32, tag="pv1")   # [s3, s4, qb3, qb4]
            v.reduce_sum(pv1[:], pt1[:].rearrange("p (g r) -> p g r", r=4),
                         axis=mybir.AxisListType.X)
            pt2 = sp.tile([128, 8], F32, tag="pt2")
            v.tensor_tensor(pt2[:].rearrange("p (g r) -> p g r", r=4),
                            T2vc[:].unsqueeze(1).broadcast_to((128, 2, 4)),
                            cw[:, C_VT2:C_VT2 + 8].rearrange(
                                "p (g r) -> p g r", r=4), ALU.mult)
            pv2 = sp.tile([128, 2], F32, tag="pv2")   # [q3, q4]
            v.reduce_sum(pv2[:], pt2[:].rearrange("p (g r) -> p g r", r=4),
                         axis=mybir.AxisListType.X)
            ps_v1 = psp.tile([128, 4], F32, tag="ps_v1")
            nc.tensor.matmul(ps_v1[:], ones[:], pv1[:])
            ps_v2 = psp.tile([128, 2], F32, tag="ps_v2")
            nc.tensor.matmul(ps_v2[:], ones[:], pv2[:])
            inv34, muinv34 = _derive_invs(nc, sp, magic, ps_v1[:, 0:2],
                                          ps_v2[:, 0:2], ps_v1[:, 2:4],
                                          imms[1], "v")
            A3p, B3p = _coef_pair(nc, sp, cw, C_W3GM, inv34[:, 0:1],
                                  muinv34[:, 0:1], has_be[2], "s", v=g)
            A4p, B4p = _coef_pair(nc, sp, cw, C_W4G4, inv34[:, 1:2],
                                  muinv34[:, 1:2], has_be[3], "k", v=g)
            # softmax stabilizer: bias bE = -VBOUND*|A3| (exact; |v|<VBOUND)
            VBOUND = 12.0
            aA3 = sp.tile([128, 2], F32, tag="aA3")
            v.tensor_scalar(aA3[:, 0:1], A3p[:, 0:1], -1.0, A3p[:, 0:1],
                            ALU.mult, ALU.max)
            v.tensor_scalar(aA3[:, 1:2], A3p[:, 1:2], -1.0, A3p[:, 1:2],
                            ALU.mult, ALU.max)
            bEp = sp.tile([128, 2], F32, tag="bEp")
            v.tensor_scalar(bEp[:], aA3[:], -VBOUND, None, ALU.mult)

            # EXPs in the window (ACT), right after the audio squares
            Es, ses = [], []
            for j in range(2):
                E = scrp.tile([128, Tv], F32, tag=f"E{j}")
                se = sp.tile([128, 1], F32, tag=f"se{j}")
                nc.scalar.activation(E[:], vf[:, Tv * j:Tv * (j + 1)],
                                     AF.Exp, bias=bEp[:, j:j + 1],
                                     scale=A3p[:, j:j + 1], accum_out=se[:])
                Es.append(E)
                ses.append(se)

            # W_i for the fusion tail: W0 raw (inv2 carried by gs0), W1 folded
            # later once inv12 is known.
            W0 = sp.tile([128, Tv], F32, tag="W0")
            nc.scalar.activation(W0[:], vf[:, 0:Tv], AF.Identity,
                                 bias=B4p[:, 0:1], scale=A4p[:, 0:1])

            audio_chunk(2)
            audio_chunk(3)
            audio_chunk(4)

            # ---- mu + variance chain: one partition-reduce matmul for all of
            # [Pmu(2) | Pq(2..4)]; mu12 read out of PSUM by the ACT engine.
            T1 = sp.tile([128, 1], F32, tag="T1")
            v.reduce_sum(T1[:], SA[:], axis=mybir.AxisListType.X)
            T2 = sp.tile([128, 1], F32, tag="T2")
            v.reduce_sum(T2[:], T2c[:], axis=mybir.AxisListType.X)
            nq = 4 if qb_zero else 6
            P4 = sp.tile([128, nq], F32, tag="P4")
            g.tensor_tensor(P4[:, 0:2], T1[:].broadcast_to((128, 2)),
                            cw[:, C_W1S:C_W1S + 2], ALU.mult)
            g.tensor_tensor(P4[:, 2:4], T2[:].broadcast_to((128, 2)),
                            cw[:, C_W1SQ:C_W1SQ + 2], ALU.mult)
            if not qb_zero:
                g.tensor_tensor(P4[:, 4:6], T1[:].broadcast_to((128, 2)),
                                cw[:, C_WB1:C_WB1 + 2], ALU.mult)
            ps_a = psp.tile([128, nq], F32, tag="ps_a")
            nc.tensor.matmul(ps_a[:], ones[:], P4[:])
            invN1, mu1_add, q1_add, _, mu2_add, q2_add = imms[0]
            mu12 = sp.tile([128, 2], F32, tag="mu12")
            nc.scalar.activation(mu12[:, 0:1], ps_a[:, 0:1], AF.Identity,
                                 bias=mu1_add, scale=invN1)
            nc.scalar.activation(mu12[:, 1:2], ps_a[:, 1:2], AF.Identity,
                                 bias=mu2_add, scale=invN1)
            # residue-1 ACT gate coefficients (inv2 factored out, p2_be==0)
            be2r = sp.tile([128, 1], F32, tag="be2r")
            g.tensor_tensor(be2r[:], cw[:, C_G2 + 1:C_G2 + 2],
                            mu12[:, 1:2], ALU.mult)
            g.tensor_tensor(be2r[:], be2r[:], cw[:, C_BG2 + 1:C_BG2 + 2],
                            ALU.add)
            # residue-0 DVE gate: c0 = mu2 * (-1/w2), c0F = mu2 * (-F/w2)
            c0 = sp.tile([128, 1], F32, tag="c0")
            g.tensor_tensor(c0[:], mu12[:, 1:2], cw[:, C_WNI:C_WNI + 1],
                            ALU.mult)
            c0F = sp.tile([128, 1], F32, tag="c0F")
            g.tensor_tensor(c0F[:], mu12[:, 1:2], cw[:, C_WNIF:C_WNIF + 1],
                            ALU.mult)

            qb = None if qb_zero else ps_a[:, 4:6]
            inv12, muinv12 = _derive_invs(nc, sp, magic, mu12[:], ps_a[:, 2:4],
                                          qb, imms[0], "a", mu_ready=True)

            # residue-0 gate coefficients: K = inv2*s0, Ka = K*m1, Kb = K*m2,
            # Kc = Kb*c0F  (gs0 = Ka*U0 + Kb*SA + Kc)
            K0 = sp.tile([128, 1], F32, tag="K0")
            g.tensor_tensor(K0[:], inv12[:, 1:2], cw[:, C_S0:C_S0 + 1], ALU.mult)
            Ka = sp.tile([128, 1], F32, tag="Ka")
            g.tensor_tensor(Ka[:], K0[:], cw[:, C_M1:C_M1 + 1], ALU.mult)
            Kb = sp.tile([128, 1], F32, tag="Kb")
            g.tensor_tensor(Kb[:], K0[:], cw[:, C_M2:C_M2 + 1], ALU.mult)
            Kc = sp.tile([128, 1], F32, tag="Kc")
            g.tensor_tensor(Kc[:], Kb[:], c0F[:], ALU.mult)
            # W1 with inv2 folded (residue-1 z lacks inv2)
            A4f = sp.tile([128, 1], F32, tag="A4f")
            g.tensor_tensor(A4f[:], A4p[:, 1:2], inv12[:, 1:2], ALU.mult)
            B4f = sp.tile([128, 1], F32, tag="B4f")
            g.tensor_tensor(B4f[:], B4p[:, 1:2], inv12[:, 1:2], ALU.mult)
            W1 = sp.tile([128, Tv], F32, tag="W1")
            nc.scalar.activation(W1[:], vf[:, Tv:2 * Tv], AF.Identity,
                                 bias=B4f[:], scale=A4f[:])

            # ---- gate relus
            with nc.allow_low_precision(reason="gate sums tolerate bf16"):
                # residue 0 on DVE: z = max(x + c0, 0) in one pass, bf16 out
                z0 = zp.tile([128, Ta * F], BF16, tag="z0")
                v.tensor_scalar(z0[:], A[:], c0[:], 0.0, ALU.add, ALU.max)
                # tree-reduce z0 over f: 64 -> 8 via bf16 adds, then reduce
                t0a = zp.tile([128, Ta * 32], BF16, tag="t0a")
                z3 = z0[:].rearrange("p (t f) -> p t f", f=F)
                v.tensor_tensor(t0a[:].rearrange("p (t f) -> p t f", f=32),
                                z3[:, :, 0:32], z3[:, :, 32:64], ALU.add)
                t0b = zp.tile([128, Ta * 16], BF16, tag="t0b")
                ta3 = t0a[:].rearrange("p (t f) -> p t f", f=32)
                v.tensor_tensor(t0b[:].rearrange("p (t f) -> p t f", f=16),
                                ta3[:, :, 0:16], ta3[:, :, 16:32], ALU.add)
                t0c = zp.tile([128, Ta * 8], BF16, tag="t0c")
                tb3 = t0b[:].rearrange("p (t f) -> p t f", f=16)
                v.tensor_tensor(t0c[:].rearrange("p (t f) -> p t f", f=8),
                                tb3[:, :, 0:8], tb3[:, :, 8:16], ALU.add)
                U0 = sp.tile([128, Ta], F32, tag="U0")
                v.reduce_sum(U0[:], t0c[:].rearrange("p (t f) -> p t f", f=8),
                             axis=mybir.AxisListType.X)

                # residue 1 on ACT (2 chunks of 2048), bf16 z; DVE trees
                SG1 = sp.tile([128, Ta], F32, tag="SG1")
                RQ = 2048
                for cc in range(2):
                    z1 = zp.tile([128, RQ], BF16, tag=f"z1_{cc}")
                    nc.scalar.activation(z1[:], A[:, RQ * cc:RQ * (cc + 1)],
                                         AF.Relu, bias=be2r[:],
                                         scale=cw[:, C_W2G2 + 1:C_W2G2 + 2])
                    t1a = zp.tile([128, 32 * 32], BF16, tag=f"t1a_{cc}")
                    z13 = z1[:].rearrange("p (t f) -> p t f", f=F)
                    v.tensor_tensor(t1a[:].rearrange("p (t f) -> p t f", f=32),
                                    z13[:, :, 0:32], z13[:, :, 32:64], ALU.add)
                    t1b = zp.tile([128, 32 * 16], BF16, tag=f"t1b_{cc}")
                    ta13 = t1a[:].rearrange("p (t f) -> p t f", f=32)
                    v.tensor_tensor(t1b[:].rearrange("p (t f) -> p t f", f=16),
                                    ta13[:, :, 0:16], ta13[:, :, 16:32], ALU.add)
                    t1c = zp.tile([128, 32 * 8], BF16, tag=f"t1c_{cc}")
                    tb13 = t1b[:].rearrange("p (t f) -> p t f", f=16)
                    v.tensor_tensor(t1c[:].rearrange("p (t f) -> p t f", f=8),
                                    tb13[:, :, 0:8], tb13[:, :, 8:16], ALU.add)
                    v.reduce_sum(SG1[:, 32 * cc:32 * (cc + 1)],
                                 t1c[:].rearrange("p (t f) -> p t f", f=8),
                                 axis=mybir.AxisListType.X)

            # ---- softmax denominators and val-path coefficients
            rc0 = sp.tile([128, 1], F32, tag="rc0")
            v.reciprocal(rc0[:], ses[0][:])
            rc1 = sp.tile([128, 1], F32, tag="rc1")
            v.reciprocal(rc1[:], ses[1][:])
            # val path (fast_val): SV_i = (w1*g1)*SA + F*be1r_i; SVp = SV*rc*inv1
            be1r = sp.tile([128, 2], F32, tag="be1r")
            g.tensor_tensor(be1r[:], cw[:, C_G1:C_G1 + 2],
                            mu12[:, 0:1].broadcast_to((128, 2)), ALU.mult)
            g.tensor_tensor(be1r[:], be1r[:], cw[:, C_BG1:C_BG1 + 2], ALU.add)
            be1x = sp.tile([128, 2], F32, tag="be1x")
            g.tensor_scalar(be1x[:], be1r[:], float(F), None, ALU.mult)
            rcp = sp.tile([128, 2], F32, tag="rcp")
            g.tensor_tensor(rcp[:, 0:1], rc0[:], inv12[:, 0:1], ALU.mult)
            g.tensor_tensor(rcp[:, 1:2], rc1[:], inv12[:, 0:1], ALU.mult)

            # ---- fusion: ot_i = P_i + gs_i (x4) * W_i
            out_sb = sp.tile([128, 2 * Tv], F32, tag="out_sb")
            # gs0 pieces on ACT (Identity with per-partition scale/bias)
            gsa = sp.tile([128, Ta], F32, tag="gsa")
            nc.scalar.activation(gsa[:], SA[:], AF.Identity,
                                 bias=Kc[:], scale=Kb[:])
            u2 = sp.tile([128, Ta], F32, tag="u2")
            nc.scalar.activation(u2[:], U0[:], AF.Identity,
                                 bias=0.0, scale=Ka[:])
            gs0 = sp.tile([128, Ta], F32, tag="gs0")
            g.tensor_tensor(gs0[:], gsa[:], u2[:], ALU.add)
            for i in range(2):
                vblk = vf[:, Tv * i:Tv * (i + 1)]
                E = Es[i]
                SV = sp.tile([128, Ta], F32, tag=f"SV{i}")
                nc.scalar.activation(SV[:], SA[:], AF.Identity,
                                     bias=be1x[:, i:i + 1],
                                     scale=cw[:, C_W1G1 + i:C_W1G1 + i + 1])
                SVp = sp.tile([128, Ta], F32, tag=f"SVp{i}")
                nc.scalar.activation(SVp[:], SV[:], AF.Identity,
                                     bias=0.0, scale=rcp[:, i:i + 1])
                P = scrp.tile([128, Tv], F32, tag=f"P{i}")
                g.tensor_tensor(P[:].rearrange("p (t k) -> p t k", k=4),
                                E[:].rearrange("p (t k) -> p t k", k=4),
                                SVp[:].unsqueeze(2).broadcast_to((128, Ta, 4)),
                                ALU.mult)
                g.tensor_tensor(P[:], P[:], vblk, ALU.add)
                gs = gs0 if i == 0 else SG1
                Wt = W0 if i == 0 else W1
                eng = g if i == 0 else v
                q = scrp.tile([128, Tv], F32, tag=f"q{i}")
                eng.tensor_tensor(q[:].rearrange("p (t k) -> p t k", k=4),
                                  Wt[:].rearrange("p (t k) -> p t k", k=4),
                                  gs[:].unsqueeze(2).broadcast_to((128, Ta, 4)),
                                  ALU.mult)
                eng.tensor_tensor(out_sb[:, Tv * i:Tv * (i + 1)], q[:], P[:],
                                  ALU.add)
            nc.sync.dma_start(out_d[:], out_sb[:])
    nc.compile()
    return nc


def _prep_consts(params):
    """Host-side parameter folding -> (cw per h, imms, has_be)."""
    (p1_w, p1_b, p1_g, p1_be, p2_w, p2_b, p2_g, p2_be,
     f1_w, f1_b, f1_g, f1_be, f2_w, f2_b, f2_g, f2_be) = [
        np.asarray(params[k], dtype=np.float64) for k in (
            "p1_w", "p1_b", "p1_g", "p1_be", "p2_w", "p2_b", "p2_g", "p2_be",
            "f1_w", "f1_b", "f1_g", "f1_be", "f2_w", "f2_b", "f2_g", "f2_be")]

    def gsum(x, g):
        return x.reshape(-1, g).sum(1)

    w1s, w1sq, wb1 = gsum(p1_w, REP), gsum(p1_w ** 2, REP), gsum(2 * p1_w * p1_b, REP)
    w2s, w2sq, wb2 = gsum(p2_w, REP), gsum(p2_w ** 2, REP), gsum(2 * p2_w * p2_b, REP)
    w3s, w3sq, wb3 = gsum(f1_w, NH), gsum(f1_w ** 2, NH), gsum(2 * f1_w * f1_b, NH)

    cws = []
    for h in range(2):
        cw = np.zeros((128, NCW), np.float64)
        cw[:, C_W1S], cw[:, C_W2S] = w1s, w2s
        cw[:, C_W1SQ], cw[:, C_W2SQ] = w1sq, w2sq
        cw[:, C_WB1], cw[:, C_WB2] = wb1, wb2
        order = [2 * h, 2 * h + 1] + [r for r in range(4) if r not in (2 * h, 2 * h + 1)]
        for pos, r in enumerate(order):
            cv = 4 * np.arange(128) + r
            cw[:, C_VT1 + 0 + pos] = w3s[cv]
            cw[:, C_VT1 + 4 + pos] = f2_w[cv]
            cw[:, C_VT1 + 8 + pos] = wb3[cv]
            cw[:, C_VT1 + 12 + pos] = 2 * f2_w[cv] * f2_b[cv]
            cw[:, C_VT2 + 0 + pos] = w3sq[cv]
            cw[:, C_VT2 + 4 + pos] = f2_w[cv] ** 2
        for i in range(2):
            cv = 4 * np.arange(128) + (2 * h + i)
            cw[:, C_W2G2 + i] = (p2_w * p2_g)[cv]
            cw[:, C_BG2 + i] = (p2_b * p2_g)[cv]
            cw[:, C_G2 + i] = -p2_g[cv]
            cw[:, C_BE2 + i] = p2_be[cv]
            cw[:, C_W1G1 + i] = (p1_w * p1_g)[cv]
            cw[:, C_BG1 + i] = (p1_b * p1_g)[cv]
            cw[:, C_G1 + i] = -p1_g[cv]
            cw[:, C_BE1 + i] = p1_be[cv]
            cw[:, C_W3GM + i] = (f1_w * f1_g).reshape(Cv, NH).mean(1)[cv]
            cw[:, C_BG3M + i] = (f1_b * f1_g).reshape(Cv, NH).mean(1)[cv]
            cw[:, C_G3M + i] = -f1_g.reshape(Cv, NH).mean(1)[cv]
            cw[:, C_BE3M + i] = f1_be.reshape(Cv, NH).mean(1)[cv]
            cw[:, C_W4G4 + i] = (f2_w * f2_g)[cv]
            cw[:, C_BG4 + i] = (f2_b * f2_g)[cv]
            cw[:, C_G4 + i] = -f2_g[cv]
            cw[:, C_BE4 + i] = f2_be[cv]
        # residue-0 DVE gate columns
        cv0 = 4 * np.arange(128) + 2 * h
        s0 = (p2_w * p2_g)[cv0]
        cw[:, C_S0] = s0
        cw[:, C_M1] = np.where(s0 > 0, 1.0, -1.0)
        cw[:, C_M2] = np.where(s0 > 0, 0.0, 1.0)
        cw[:, C_WNI] = -1.0 / p2_w[cv0]
        cw[:, C_WNIF] = -float(F) / p2_w[cv0]
        cws.append(cw.astype(np.float32))

    imm_a = (1.0 / N1, Ta * F * p1_b.sum() / N1, Ta * F * (p1_b ** 2).sum() / N1 + EPS,
             1.0 / N1, Ta * F * p2_b.sum() / N1, Ta * F * (p2_b ** 2).sum() / N1 + EPS)
    imm_v = (1.0 / N3, Tv * f1_b.sum() / N3, Tv * (f1_b ** 2).sum() / N3 + EPS,
             1.0 / N4, Tv * f2_b.sum() / N4, Tv * (f2_b ** 2).sum() / N4 + EPS)
    imms = (tuple(float(x) for x in imm_a), tuple(float(x) for x in imm_v))
    has_be = (bool(np.any(p1_be)), bool(np.any(p2_be)),
              bool(np.any(f1_be)), bool(np.any(f2_be)),
              not (np.any(p1_b) or np.any(p2_b)))
    return cws, imms, has_be


def kernel(**inputs):
    global LAST_EXEC_NS, LAST_RESULTS
    audio = np.ascontiguousarray(np.asarray(inputs["audio"], dtype=np.float32))
    video = np.ascontiguousarray(np.asarray(inputs["video"], dtype=np.float32))
    cws, imms, has_be = _prep_consts(inputs)

    key = ("prog", imms, has_be)
    if key not in _CACHE:
        _CACHE[key] = build_program(imms, has_be)
    nc = _CACHE[key]

    in_maps = []
    for core in range(8):
        b, h = core // 2, core % 2
        vres = video[b].reshape(128, 4, Tv)
        order = [2 * h, 2 * h + 1] + [r for r in range(4) if r not in (2 * h, 2 * h + 1)]
        vfm = np.ascontiguousarray(vres[:, order, :].reshape(128, 4 * Tv))
        in_maps.append({
            "audio_s": np.ascontiguousarray(audio[b].reshape(128, Ta * F)),
            "video_f": vfm,
            "cw": cws[h],
        })

    trace = bool(int(os.environ.get("BASS_KERNEL_TRACE", "0")))
    res = run_bass_kernel_spmd(nc, in_maps, list(range(8)), trace=trace)
    LAST_EXEC_NS = res.exec_time_ns
    LAST_RESULTS = res
    out = np.empty((B, Cv, Tv), np.float32)
    for core in range(8):
        b, h = core // 2, core % 2
        oc = res.results[core]["out_c"].reshape(128, 2, Tv)
        ov = out[b].reshape(128, 4, Tv)
        ov[:, 2 * h, :] = oc[:, 0, :]
        ov[:, 2 * h + 1, :] = oc[:, 1, :]
    return out


# revision 11
# speedup vs baseline: 1.1004x; 1.1004x over previous
"""Trainium2 Bass kernel for nn_CAFVBlock (audio/video cross-attention fusion).

v5 strategy (8 NeuronCores, SPMD): core = 2*b + h handles sample b and output
channel residues r in {2h, 2h+1} (cv = 4*ca + r).  Built on the v1 skeleton
(ACT gate relus overlapped with DVE reductions) with:
  - relu outputs in bf16; segmented f-sums via 2x-rate bf16 tree adds + a
    short TensorReduce instead of full-rate TensorReduce (DVE -2us)
  - one merged partition-reduce matmul for [Pmu | Pq]; the relu bias be2r is
    produced directly from PSUM by ACT Identity ops with host-folded
    scale/bias columns, so the mu -> relu chain never leaves the ACT engine
  - EXPs issued in the DMA window (softmax denominators ready early)
  - fusion tail restructured as ot_i = P_i + SG_i (x4) * W_i with P/W
    precomputed on GpSimd while the gate phase runs
  - single merged output DMA
"""
import os
import sys
import numpy as np

for _p in ("/opt/trn_rl_repo",):
    if _p not in sys.path and os.path.isdir(_p):
        sys.path.insert(0, _p)

import concourse.bass as bass
import concourse.tile as tile
from concourse import bacc, mybir
from concourse.bass_utils import run_bass_kernel_spmd

F32 = mybir.dt.float32
BF16 = mybir.dt.bfloat16
I32 = mybir.dt.int32
AF = mybir.ActivationFunctionType
ALU = mybir.AluOpType
RSQRT_MAGIC = 0x5F3759DF

B, Ca, Cv, NH = 4, 128, 512, 8
Ta, F, Tv = 64, 64, 256
REP = Cv // Ca   # 4
EPS = 1e-5
N1 = Cv * Ta * F
N3 = Cv * NH * Tv
N4 = Cv * Tv

C_W1S, C_W2S, C_W1SQ, C_W2SQ, C_WB1, C_WB2 = 0, 1, 2, 3, 4, 5
C_VT1 = 6    # 16 cols: [V3S(4), V4S(4), VB3(4), VB4(4)]  (T1v-weighted)
C_VT2 = 22   # 8 cols:  [V3SQ(4), V4SQ(4)]                (T2v-weighted)
C_W2G2, C_BG2, C_G2, C_BE2 = 30, 32, 34, 36     # +i for i in {0,1}
C_W1G1, C_BG1, C_G1, C_BE1 = 38, 40, 42, 44
C_W3GM, C_BG3M, C_G3M, C_BE3M = 46, 48, 50, 52
C_W4G4, C_BG4, C_G4, C_BE4 = 54, 56, 58, 60
C_BERS, C_BERB = 62, 64                          # be2r from PSUM: scale,bias
NCW = 66

_CACHE = {}
LAST_EXEC_NS = None
LAST_RESULTS = None


def _derive_invs(nc, sp, magic, s_ap, q_ap, qb_ap, imms, tag, mu_ready=False,
                 n_iter=2):
    v = nc.vector
    invN_a, mua_a, qa_a, invN_b, mua_b, qa_b = imms
    if mu_ready:
        mu_ap = s_ap
    else:
        mu = sp.tile([128, 2], F32, tag=f"mu{tag}")
        v.tensor_scalar(mu[:, 0:1], s_ap[:, 0:1], invN_a, mua_a, ALU.mult, ALU.add)
        v.tensor_scalar(mu[:, 1:2], s_ap[:, 1:2], invN_b, mua_b, ALU.mult, ALU.add)
        mu_ap = mu[:]
    if qb_ap is not None:
        qbs = sp.tile([128, 2], F32, tag=f"qbs{tag}")
        v.tensor_copy(qbs[:], qb_ap)
        qs = sp.tile([128, 2], F32, tag=f"qs{tag}")
        v.tensor_tensor(qs[:], q_ap, qbs[:], ALU.add)
        qs_ap = qs[:]
    else:
        qs_ap = q_ap
    qn = sp.tile([128, 2], F32, tag=f"qn{tag}")
    v.tensor_scalar(qn[:, 0:1], qs_ap[:, 0:1], invN_a, qa_a, ALU.mult, ALU.add)
    v.tensor_scalar(qn[:, 1:2], qs_ap[:, 1:2], invN_b, qa_b, ALU.mult, ALU.add)
    mm = sp.tile([128, 2], F32, tag=f"mm{tag}")
    v.tensor_tensor(mm[:], mu_ap, mu_ap, ALU.mult)
    varp = sp.tile([128, 2], F32, tag=f"varp{tag}")
    v.tensor_tensor(varp[:], qn[:], mm[:], ALU.subtract)
    half = sp.tile([128, 2], I32, tag=f"half{tag}")
    v.tensor_scalar(half[:], varp[:].bitcast(I32), 1, None, ALU.arith_shift_right)
    yi = sp.tile([128, 2], I32, tag=f"yi{tag}")
    v.tensor_tensor(yi[:], magic[:, 0:2], half[:], ALU.subtract)
    xh = sp.tile([128, 2], F32, tag=f"xh{tag}")
    v.tensor_scalar(xh[:], varp[:], 0.5, None, ALU.mult)
    y = yi[:].bitcast(F32)
    for it in range(n_iter):
        t2 = sp.tile([128, 2], F32, tag=f"t2{tag}{it}")
        v.tensor_tensor(t2[:], y, y, ALU.mult)
        v.tensor_tensor(t2[:], t2[:], xh[:], ALU.mult)
        v.tensor_scalar(t2[:], t2[:], -1.0, 1.5, ALU.mult, ALU.add)
        yn = sp.tile([128, 2], F32, tag=f"yn{tag}{it}")
        v.tensor_tensor(yn[:], y, t2[:], ALU.mult)
        y = yn[:]
    inv = y
    muinv = sp.tile([128, 2], F32, tag=f"muinv{tag}")
    v.tensor_tensor(muinv[:], mu_ap, inv, ALU.mult)
    return inv, muinv


def _coef_pair(nc, sp, cw, base, inv_ap, muinv_ap, has_be, tag, v=None):
    if v is None:
        v = nc.vector
    invb = inv_ap.broadcast_to((128, 2))
    alpha = sp.tile([128, 2], F32, tag=f"al{tag}")
    v.tensor_tensor(alpha[:], cw[:, base:base + 2], invb, ALU.mult)
    beta = sp.tile([128, 2], F32, tag=f"be{tag}")
    v.tensor_tensor(beta[:], cw[:, base + 2:base + 4], invb, ALU.mult)
    tb = sp.tile([128, 2], F32, tag=f"tb{tag}")
    v.tensor_tensor(tb[:], cw[:, base + 4:base + 6],
                    muinv_ap.broadcast_to((128, 2)), ALU.mult)
    v.tensor_tensor(beta[:], beta[:], tb[:], ALU.add)
    if has_be:
        v.tensor_tensor(beta[:], beta[:], cw[:, base + 6:base + 8], ALU.add)
    return alpha, beta


def build_program(imms, has_be):
    nc = bacc.Bacc("TRN2", target_bir_lowering=False, debug=False, num_devices=8)

    audio_s = nc.dram_tensor("audio_s", [128, Ta * F], F32, kind="ExternalInput")
    video_f = nc.dram_tensor("video_f", [128, REP * Tv], F32, kind="ExternalInput")
    cw_d = nc.dram_tensor("cw", [128, NCW], F32, kind="ExternalInput")
    out_d = nc.dram_tensor("out_c", [128, 2 * Tv], F32, kind="ExternalOutput")

    offs = [0, 1024, 2048, 2560, 3072, 3584]
    sizes = [1024, 1024, 512, 512, 512, 512]
    qb_zero = has_be[4] if len(has_be) > 4 else False
    fast_gate = not has_be[1]
    fast_val = not has_be[0]
    assert fast_gate and fast_val, "v5 assumes p1_b=p2_b=0, p1_be=p2_be=0"

    with tile.TileContext(nc) as tc:
        with (
            tc.tile_pool(name="big", bufs=1) as bigp,
            tc.tile_pool(name="z", bufs=2) as zp,
            tc.tile_pool(name="scr", bufs=2) as scrp,
            tc.tile_pool(name="sp", bufs=1) as sp,
            tc.tile_pool(name="psum", bufs=2, space="PSUM") as psp,
        ):
            v = nc.vector
            g = nc.gpsimd
            A = bigp.tile([128, Ta * F], F32, tag="A")
            vf = bigp.tile([128, REP * Tv], F32, tag="vf")
            cw = bigp.tile([128, NCW], F32, tag="cw")
            ones = bigp.tile([128, 128], F32, tag="ones")
            magic = bigp.tile([128, 2], I32, tag="magic")

            # ---- input DMAs (v1 ordering: small tensors first so the video
            # chain runs inside the audio window; audio split across rings)
            VH = REP * Tv // 2
            nc.sync.dma_start(vf[:, :VH], video_f[:, :VH])
            nc.scalar.dma_start(cw[:], cw_d[:])
            nc.scalar.dma_start(vf[:, VH:], video_f[:, VH:])
            dma_eng = [nc.sync, nc.scalar]
            for c in range(6):
                dma_eng[c % 2].dma_start(A[:, offs[c]:offs[c] + sizes[c]],
                                         audio_s[:, offs[c]:offs[c] + sizes[c]])
            g.memset(ones[:], 1.0)
            g.memset(magic[:], RSQRT_MAGIC)

            # ---- video stats per half
            T2vc = sp.tile([128, 4], F32, tag="T2vc")
            T1vc = sp.tile([128, 4], F32, tag="T1vc")
            for hh in range(2):
                hs = slice(VH * hh, VH * (hh + 1))
                v.reduce_sum(T1vc[:, 2 * hh:2 * hh + 2],
                             vf[:, hs].rearrange("p (r t) -> p r t", t=Tv),
                             axis=mybir.AxisListType.X)
                vsq = scrp.tile([128, VH], F32, tag="vsq")
                nc.scalar.activation(vsq[:], vf[:, hs], AF.Square)
                v.reduce_sum(T2vc[:, 2 * hh:2 * hh + 2],
                             vsq[:].rearrange("p (r t) -> p r t", t=Tv),
                             axis=mybir.AxisListType.X)
            pt1 = sp.tile([128, 16], F32, tag="pt1")
            v.tensor_tensor(pt1[:].rearrange("p (g r) -> p g r", r=4),
                            T1vc[:].unsqueeze(1).broadcast_to((128, 4, 4)),
                            cw[:, C_VT1:C_VT1 + 16].rearrange(
                                "p (g r) -> p g r", r=4), ALU.mult)
            pv1 = sp.tile([128, 4], F32, tag="pv1")
            v.reduce_sum(pv1[:], pt1[:].rearrange("p (g r) -> p g r", r=4),
                         axis=mybir.AxisListType.X)
            pt2 = sp.tile([128, 8], F32, tag="pt2")
            v.tensor_tensor(pt2[:].rearrange("p (g r) -> p g r", r=4),
                            T2vc[:].unsqueeze(1).broadcast_to((128, 2, 4)),
                            cw[:, C_VT2:C_VT2 + 8].rearrange(
                                "p (g r) -> p g r", r=4), ALU.mult)
            pv2 = sp.tile([128, 2], F32, tag="pv2")
            v.reduce_sum(pv2[:], pt2[:].rearrange("p (g r) -> p g r", r=4),
                         axis=mybir.AxisListType.X)
            ps_v1 = psp.tile([128, 4], F32, tag="ps_v1")
            nc.tensor.matmul(ps_v1[:], ones[:], pv1[:])
            ps_v2 = psp.tile([128, 2], F32, tag="ps_v2")
            nc.tensor.matmul(ps_v2[:], ones[:], pv2[:])
            inv34, muinv34 = _derive_invs(nc, sp, magic, ps_v1[:, 0:2],
                                          ps_v2[:, 0:2], ps_v1[:, 2:4],
                                          imms[1], "v")
            A3p, B3p = _coef_pair(nc, sp, cw, C_W3GM, inv34[:, 0:1],
                                  muinv34[:, 0:1], has_be[2], "s", v=g)
            A4p, B4p = _coef_pair(nc, sp, cw, C_W4G4, inv34[:, 1:2],
                                  muinv34[:, 1:2], has_be[3], "k", v=g)
            VBOUND = 12.0
            aA3 = sp.tile([128, 2], F32, tag="aA3")
            v.tensor_scalar(aA3[:, 0:1], A3p[:, 0:1], -1.0, A3p[:, 0:1],
                            ALU.mult, ALU.max)
            v.tensor_scalar(aA3[:, 1:2], A3p[:, 1:2], -1.0, A3p[:, 1:2],
                            ALU.mult, ALU.max)
            bEp = sp.tile([128, 2], F32, tag="bEp")
            v.tensor_scalar(bEp[:], aA3[:], -VBOUND, None, ALU.mult)

            # EXPs in the DMA window (denominators ready before the fusion)
            Es, ses = [], []
            for j in range(2):
                E = scrp.tile([128, Tv], F32, tag=f"E{j}")
                se = sp.tile([128, 1], F32, tag=f"se{j}")
                nc.scalar.activation(E[:], vf[:, Tv * j:Tv * (j + 1)],
                                     AF.Exp, bias=bEp[:, j:j + 1],
                                     scale=A3p[:, j:j + 1], accum_out=se[:])
                Es.append(E)
                ses.append(se)
            rc0 = sp.tile([128, 1], F32, tag="rc0")
            v.reciprocal(rc0[:], ses[0][:])
            rc1 = sp.tile([128, 1], F32, tag="rc1")
            v.reciprocal(rc1[:], ses[1][:])

            # ---- audio scans per chunk
            SA = sp.tile([128, Ta], F32, tag="SA")
            T2c = sp.tile([128, 6], F32, tag="T2c")
            for c in range(6):
                v.reduce_sum(SA[:, offs[c] // F:(offs[c] + sizes[c]) // F],
                             A[:, offs[c]:offs[c] + sizes[c]].rearrange(
                                 "p (t f) -> p t f", f=F),
                             axis=mybir.AxisListType.X)
                sq = scrp.tile([128, 1024], F32, tag="sq")
                nc.scalar.activation(sq[:, :sizes[c]],
                                     A[:, offs[c]:offs[c] + sizes[c]], AF.Square,
                                     accum_out=T2c[:, c:c + 1])

            # ---- merged mu + variance partition reduce
            T1 = sp.tile([128, 1], F32, tag="T1")
            v.reduce_sum(T1[:], SA[:], axis=mybir.AxisListType.X)
            T2 = sp.tile([128, 1], F32, tag="T2")
            v.reduce_sum(T2[:], T2c[:], axis=mybir.AxisListType.X)
            nq = 4 if qb_zero else 6
            P4 = sp.tile([128, nq], F32, tag="P4")
            v.tensor_tensor(P4[:, 0:2], T1[:].broadcast_to((128, 2)),
                            cw[:, C_W1S:C_W1S + 2], ALU.mult)
            v.tensor_tensor(P4[:, 2:4], T2[:].broadcast_to((128, 2)),
                            cw[:, C_W1SQ:C_W1SQ + 2], ALU.mult)
            if not qb_zero:
                v.tensor_tensor(P4[:, 4:6], T1[:].broadcast_to((128, 2)),
                                cw[:, C_WB1:C_WB1 + 2], ALU.mult)
            ps_a = psp.tile([128, nq], F32, tag="ps_a")
            nc.tensor.matmul(ps_a[:], ones[:], P4[:])
            invN1, mu1_add, q1_add, _, mu2_add, q2_add = imms[0]
            # relu bias be2r_i straight from PSUM on ACT (host-folded cols)
            be2r = sp.tile([128, 2], F32, tag="be2r")
            for i in range(2):
                nc.scalar.activation(be2r[:, i:i + 1], ps_a[:, 1:2], AF.Identity,
                                     bias=cw[:, C_BERB + i:C_BERB + i + 1],
                                     scale=cw[:, C_BERS + i:C_BERS + i + 1])
            mu12 = sp.tile([128, 2], F32, tag="mu12")
            v.tensor_scalar(mu12[:, 0:1], ps_a[:, 0:1], invN1, mu1_add,
                            ALU.mult, ALU.add)
            v.tensor_scalar(mu12[:, 1:2], ps_a[:, 1:2], invN1, mu2_add,
                            ALU.mult, ALU.add)
            qb = None if qb_zero else ps_a[:, 4:6]
            inv12, muinv12 = _derive_invs(nc, sp, magic, mu12[:], ps_a[:, 2:4],
                                          qb, imms[0], "a", mu_ready=True)

            # gate fold: A4pp/B4pp = A4p/B4p * inv2 (both residues)
            A4pp = sp.tile([128, 2], F32, tag="A4pp")
            g.tensor_tensor(A4pp[:], A4p[:],
                            inv12[:, 1:2].broadcast_to((128, 2)), ALU.mult)
            B4pp = sp.tile([128, 2], F32, tag="B4pp")
            g.tensor_tensor(B4pp[:], B4p[:],
                            inv12[:, 1:2].broadcast_to((128, 2)), ALU.mult)
            # W_i = A4pp_i * v + B4pp_i  (GpSimd TT pairs, in parallel with relus)
            Ws = []
            for i in range(2):
                W = scrp.tile([128, Tv], F32, tag=f"W{i}")
                g.tensor_tensor(W[:], vf[:, Tv * i:Tv * (i + 1)],
                                A4pp[:, i:i + 1].broadcast_to((128, Tv)),
                                ALU.mult)
                g.tensor_tensor(W[:], W[:],
                                B4pp[:, i:i + 1].broadcast_to((128, Tv)),
                                ALU.add)
                Ws.append(W)
            # val-path folded coefficients: SVp_i = sc_i*SA + bi_i
            be1r = sp.tile([128, 2], F32, tag="be1r")
            g.tensor_tensor(be1r[:], cw[:, C_G1:C_G1 + 2],
                            mu12[:, 0:1].broadcast_to((128, 2)), ALU.mult)
            g.tensor_tensor(be1r[:], be1r[:], cw[:, C_BG1:C_BG1 + 2], ALU.add)
            rcp = sp.tile([128, 2], F32, tag="rcp")
            g.tensor_tensor(rcp[:, 0:1], rc0[:], inv12[:, 0:1], ALU.mult)
            g.tensor_tensor(rcp[:, 1:2], rc1[:], inv12[:, 0:1], ALU.mult)
            svs = sp.tile([128, 2], F32, tag="svs")
            g.tensor_tensor(svs[:], rcp[:], cw[:, C_W1G1:C_W1G1 + 2], ALU.mult)
            svb = sp.tile([128, 2], F32, tag="svb")
            g.tensor_tensor(svb[:], rcp[:], be1r[:], ALU.mult)
            g.tensor_scalar(svb[:], svb[:], float(F), None, ALU.mult)

            # ---- gate relus (ACT, bf16 z) + DVE tree sums
            SG = sp.tile([128, 2 * Ta], F32, tag="SG")
            RQ = 2048
            with nc.allow_low_precision(reason="gate sums tolerate bf16"):
                for i in range(2):
                    for cc in range(2):
                        z1 = zp.tile([128, RQ], BF16, tag=f"z{i}{cc}")
                        nc.scalar.activation(
                            z1[:], A[:, RQ * cc:RQ * (cc + 1)], AF.Relu,
                            bias=be2r[:, i:i + 1],
                            scale=cw[:, C_W2G2 + i:C_W2G2 + i + 1])
                        t1a = zp.tile([128, 32 * 32], BF16, tag=f"ta{i}{cc}")
                        z13 = z1[:].rearrange("p (t f) -> p t f", f=F)
                        v.tensor_tensor(t1a[:].rearrange("p (t f) -> p t f", f=32),
                                        z13[:, :, 0:32], z13[:, :, 32:64], ALU.add)
                        t1b = zp.tile([128, 32 * 16], BF16, tag=f"tb{i}{cc}")
                        ta13 = t1a[:].rearrange("p (t f) -> p t f", f=32)
                        v.tensor_tensor(t1b[:].rearrange("p (t f) -> p t f", f=16),
                                        ta13[:, :, 0:16], ta13[:, :, 16:32], ALU.add)
                        t1c = zp.tile([128, 32 * 8], BF16, tag=f"tc{i}{cc}")
                        tb13 = t1b[:].rearrange("p (t f) -> p t f", f=16)
                        v.tensor_tensor(t1c[:].rearrange("p (t f) -> p t f", f=8),
                                        tb13[:, :, 0:8], tb13[:, :, 8:16], ALU.add)
                        v.reduce_sum(SG[:, Ta * i + 32 * cc:Ta * i + 32 * (cc + 1)],
                                     t1c[:].rearrange("p (t f) -> p t f", f=8),
                                     axis=mybir.AxisListType.X)

            # ---- fusion: ot_i = P_i + SG_i (x4) * W_i
            out_sb = sp.tile([128, 2 * Tv], F32, tag="out_sb")
            for i in range(2):
                vblk = vf[:, Tv * i:Tv * (i + 1)]
                SVp = sp.tile([128, Ta], F32, tag=f"SVp{i}")
                nc.scalar.activation(SVp[:], SA[:], AF.Identity,
                                     bias=svb[:, i:i + 1],
                                     scale=svs[:, i:i + 1])
                P = scrp.tile([128, Tv], F32, tag=f"P{i}")
                g.tensor_tensor(P[:].rearrange("p (t k) -> p t k", k=4),
                                Es[i][:].rearrange("p (t k) -> p t k", k=4),
                                SVp[:].unsqueeze(2).broadcast_to((128, Ta, 4)),
                                ALU.mult)
                g.tensor_tensor(P[:], P[:], vblk, ALU.add)
                sg_blk = SG[:, Ta * i:Ta * (i + 1)]
                eng = g if i == 0 else v
                q = scrp.tile([128, Tv], F32, tag=f"q{i}")
                eng.tensor_tensor(q[:].rearrange("p (t k) -> p t k", k=4),
                                  Ws[i][:].rearrange("p (t k) -> p t k", k=4),
                                  sg_blk.unsqueeze(2).broadcast_to((128, Ta, 4)),
                                  ALU.mult)
                eng.tensor_tensor(out_sb[:, Tv * i:Tv * (i + 1)], q[:], P[:],
                                  ALU.add)
            nc.sync.dma_start(out_d[:], out_sb[:])
    nc.compile()
    return nc


def _prep_consts(params):
    (p1_w, p1_b, p1_g, p1_be, p2_w, p2_b, p2_g, p2_be,
     f1_w, f1_b, f1_g, f1_be, f2_w, f2_b, f2_g, f2_be) = [
        np.asarray(params[k], dtype=np.float64) for k in (
            "p1_w", "p1_b", "p1_g", "p1_be", "p2_w", "p2_b", "p2_g", "p2_be",
            "f1_w", "f1_b", "f1_g", "f1_be", "f2_w", "f2_b", "f2_g", "f2_be")]

    def gsum(x, g):
        return x.reshape(-1, g).sum(1)

    w1s, w1sq, wb1 = gsum(p1_w, REP), gsum(p1_w ** 2, REP), gsum(2 * p1_w * p1_b, REP)
    w2s, w2sq, wb2 = gsum(p2_w, REP), gsum(p2_w ** 2, REP), gsum(2 * p2_w * p2_b, REP)
    w3s, w3sq, wb3 = gsum(f1_w, NH), gsum(f1_w ** 2, NH), gsum(2 * f1_w * f1_b, NH)

    invN1 = 1.0 / N1
    mu2_add = Ta * F * p2_b.sum() / N1

    cws = []
    for h in range(2):
        cw = np.zeros((128, NCW), np.float64)
        cw[:, C_W1S], cw[:, C_W2S] = w1s, w2s
        cw[:, C_W1SQ], cw[:, C_W2SQ] = w1sq, w2sq
        cw[:, C_WB1], cw[:, C_WB2] = wb1, wb2
        order = [2 * h, 2 * h + 1] + [r for r in range(4) if r not in (2 * h, 2 * h + 1)]
        for pos, r in enumerate(order):
            cv = 4 * np.arange(128) + r
            cw[:, C_VT1 + 0 + pos] = w3s[cv]
            cw[:, C_VT1 + 4 + pos] = f2_w[cv]
            cw[:, C_VT1 + 8 + pos] = wb3[cv]
            cw[:, C_VT1 + 12 + pos] = 2 * f2_w[cv] * f2_b[cv]
            cw[:, C_VT2 + 0 + pos] = w3sq[cv]
            cw[:, C_VT2 + 4 + pos] = f2_w[cv] ** 2
        for i in range(2):
            cv = 4 * np.arange(128) + (2 * h + i)
            cw[:, C_W2G2 + i] = (p2_w * p2_g)[cv]
            cw[:, C_BG2 + i] = (p2_b * p2_g)[cv]
            cw[:, C_G2 + i] = -p2_g[cv]
            cw[:, C_BE2 + i] = p2_be[cv]
            cw[:, C_W1G1 + i] = (p1_w * p1_g)[cv]
            cw[:, C_BG1 + i] = (p1_b * p1_g)[cv]
            cw[:, C_G1 + i] = -p1_g[cv]
            cw[:, C_BE1 + i] = p1_be[cv]
            cw[:, C_W3GM + i] = (f1_w * f1_g).reshape(Cv, NH).mean(1)[cv]
            cw[:, C_BG3M + i] = (f1_b * f1_g).reshape(Cv, NH).mean(1)[cv]
            cw[:, C_G3M + i] = -f1_g.reshape(Cv, NH).mean(1)[cv]
            cw[:, C_BE3M + i] = f1_be.reshape(Cv, NH).mean(1)[cv]
            cw[:, C_W4G4 + i] = (f2_w * f2_g)[cv]
            cw[:, C_BG4 + i] = (f2_b * f2_g)[cv]
            cw[:, C_G4 + i] = -f2_g[cv]
            cw[:, C_BE4 + i] = f2_be[cv]
            # be2r_i = (-g2*invN1)*ps2 + (-g2*mu2_add + b2*g2)
            cw[:, C_BERS + i] = (-p2_g[cv]) * invN1
            cw[:, C_BERB + i] = (-p2_g[cv]) * mu2_add + (p2_b * p2_g)[cv]
        cws.append(cw.astype(np.float32))

    imm_a = (invN1, Ta * F * p1_b.sum() / N1, Ta * F * (p1_b ** 2).sum() / N1 + EPS,
             invN1, mu2_add, Ta * F * (p2_b ** 2).sum() / N1 + EPS)
    imm_v = (1.0 / N3, Tv * f1_b.sum() / N3, Tv * (f1_b ** 2).sum() / N3 + EPS,
             1.0 / N4, Tv * f2_b.sum() / N4, Tv * (f2_b ** 2).sum() / N4 + EPS)
    imms = (tuple(float(x) for x in imm_a), tuple(float(x) for x in imm_v))
    has_be = (bool(np.any(p1_be)), bool(np.any(p2_be)),
              bool(np.any(f1_be)), bool(np.any(f2_be)),
              not (np.any(p1_b) or np.any(p2_b)))
    return cws, imms, has_be


def kernel(**inputs):
    global LAST_EXEC_NS, LAST_RESULTS
    audio = np.ascontiguousarray(np.asarray(inputs["audio"], dtype=np.float32))
    video = np.ascontiguousarray(np.asarray(inputs["video"], dtype=np.float32))
    cws, imms, has_be = _prep_consts(inputs)

    key = ("prog", imms, has_be)
    if key not in _CACHE:
        _CACHE[key] = build_program(imms, has_be)
    nc = _CACHE[key]

    in_maps = []
    for core in range(8):
        b, h = core // 2, core % 2
        vres = video[b].reshape(128, 4, Tv)
        order = [2 * h, 2 * h + 1] + [r for r in range(4) if r not in (2 * h, 2 * h + 1)]
        vfm = np.ascontiguousarray(vres[:, order, :].reshape(128, 4 * Tv))
        in_maps.append({
            "audio_s": np.ascontiguousarray(audio[b].reshape(128, Ta * F)),
            "video_f": vfm,
            "cw": cws[h],
        })

    trace = bool(int(os.environ.get("BASS_KERNEL_TRACE", "0")))
    res = run_bass_kernel_spmd(nc, in_maps, list(range(8)), trace=trace)
    LAST_EXEC_NS = res.exec_time_ns
    LAST_RESULTS = res
    out = np.empty((B, Cv, Tv), np.float32)
    for core in range(8):
        b, h = core // 2, core % 2
        oc = res.results[core]["out_c"].reshape(128, 2, Tv)
        ov = out[b].reshape(128, 4, Tv)
        ov[:, 2 * h, :] = oc[:, 0, :]
        ov[:, 2 * h + 1, :] = oc[:, 1, :]
    return out


# revision 15
# speedup vs baseline: 1.1082x; 1.0070x over previous
"""Trainium2 Bass kernel for nn_CAFVBlock (audio/video cross-attention fusion).

v5 strategy (8 NeuronCores, SPMD): core = 2*b + h handles sample b and output
channel residues r in {2h, 2h+1} (cv = 4*ca + r).  Built on the v1 skeleton
(ACT gate relus overlapped with DVE reductions) with:
  - relu outputs in bf16; segmented f-sums via 2x-rate bf16 tree adds + a
    short TensorReduce instead of full-rate TensorReduce (DVE -2us)
  - one merged partition-reduce matmul for [Pmu | Pq]; the relu bias be2r is
    produced directly from PSUM by ACT Identity ops with host-folded
    scale/bias columns, so the mu -> relu chain never leaves the ACT engine
  - EXPs issued in the DMA window (softmax denominators ready early)
  - fusion tail restructured as ot_i = P_i + SG_i (x4) * W_i with P/W
    precomputed on GpSimd while the gate phase runs
  - single merged output DMA
"""
import os
import sys
import numpy as np

for _p in ("/opt/trn_rl_repo",):
    if _p not in sys.path and os.path.isdir(_p):
        sys.path.insert(0, _p)

import concourse.bass as bass
import concourse.tile as tile
from concourse import bacc, mybir
from concourse.bass_utils import run_bass_kernel_spmd

F32 = mybir.dt.float32
BF16 = mybir.dt.bfloat16
I32 = mybir.dt.int32
AF = mybir.ActivationFunctionType
ALU = mybir.AluOpType
RSQRT_MAGIC = 0x5F3759DF

B, Ca, Cv, NH = 4, 128, 512, 8
Ta, F, Tv = 64, 64, 256
REP = Cv // Ca   # 4
EPS = 1e-5
N1 = Cv * Ta * F
N3 = Cv * NH * Tv
N4 = Cv * Tv

C_W1S, C_W2S, C_W1SQ, C_W2SQ, C_WB1, C_WB2 = 0, 1, 2, 3, 4, 5
C_VT1 = 6    # 16 cols: [V3S(4), V4S(4), VB3(4), VB4(4)]  (T1v-weighted)
C_VT2 = 22   # 8 cols:  [V3SQ(4), V4SQ(4)]                (T2v-weighted)
C_W2G2, C_BG2, C_G2, C_BE2 = 30, 32, 34, 36     # +i for i in {0,1}
C_W1G1, C_BG1, C_G1, C_BE1 = 38, 40, 42, 44
C_W3GM, C_BG3M, C_G3M, C_BE3M = 46, 48, 50, 52
C_W4G4, C_BG4, C_G4, C_BE4 = 54, 56, 58, 60
C_BERS, C_BERB = 62, 64                          # be2r from PSUM: scale,bias
NCW = 66

_CACHE = {}
LAST_EXEC_NS = None
LAST_RESULTS = None


def _derive_invs(nc, sp, magic, s_ap, q_ap, qb_ap, imms, tag, mu_ready=False,
                 n_iter=2):
    v = nc.vector
    invN_a, mua_a, qa_a, invN_b, mua_b, qa_b = imms
    if mu_ready:
        mu_ap = s_ap
    else:
        mu = sp.tile([128, 2], F32, tag=f"mu{tag}")
        v.tensor_scalar(mu[:, 0:1], s_ap[:, 0:1], invN_a, mua_a, ALU.mult, ALU.add)
        v.tensor_scalar(mu[:, 1:2], s_ap[:, 1:2], invN_b, mua_b, ALU.mult, ALU.add)
        mu_ap = mu[:]
    if qb_ap is not None:
        qbs = sp.tile([128, 2], F32, tag=f"qbs{tag}")
        v.tensor_copy(qbs[:], qb_ap)
        qs = sp.tile([128, 2], F32, tag=f"qs{tag}")
        v.tensor_tensor(qs[:], q_ap, qbs[:], ALU.add)
        qs_ap = qs[:]
    else:
        qs_ap = q_ap
    qn = sp.tile([128, 2], F32, tag=f"qn{tag}")
    v.tensor_scalar(qn[:, 0:1], qs_ap[:, 0:1], invN_a, qa_a, ALU.mult, ALU.add)
    v.tensor_scalar(qn[:, 1:2], qs_ap[:, 1:2], invN_b, qa_b, ALU.mult, ALU.add)
    mm = sp.tile([128, 2], F32, tag=f"mm{tag}")
    v.tensor_tensor(mm[:], mu_ap, mu_ap, ALU.mult)
    varp = sp.tile([128, 2], F32, tag=f"varp{tag}")
    v.tensor_tensor(varp[:], qn[:], mm[:], ALU.subtract)
    half = sp.tile([128, 2], I32, tag=f"half{tag}")
    v.tensor_scalar(half[:], varp[:].bitcast(I32), 1, None, ALU.arith_shift_right)
    yi = sp.tile([128, 2], I32, tag=f"yi{tag}")
    v.tensor_tensor(yi[:], magic[:, 0:2], half[:], ALU.subtract)
    xh = sp.tile([128, 2], F32, tag=f"xh{tag}")
    v.tensor_scalar(xh[:], varp[:], 0.5, None, ALU.mult)
    y = yi[:].bitcast(F32)
    for it in range(n_iter):
        t2 = sp.tile([128, 2], F32, tag=f"t2{tag}{it}")
        v.tensor_tensor(t2[:], y, y, ALU.mult)
        v.tensor_tensor(t2[:], t2[:], xh[:], ALU.mult)
        v.tensor_scalar(t2[:], t2[:], -1.0, 1.5, ALU.mult, ALU.add)
        yn = sp.tile([128, 2], F32, tag=f"yn{tag}{it}")
        v.tensor_tensor(yn[:], y, t2[:], ALU.mult)
        y = yn[:]
    inv = y
    muinv = sp.tile([128, 2], F32, tag=f"muinv{tag}")
    v.tensor_tensor(muinv[:], mu_ap, inv, ALU.mult)
    return inv, muinv


def _coef_pair(nc, sp, cw, base, inv_ap, muinv_ap, has_be, tag, v=None):
    if v is None:
        v = nc.vector
    invb = inv_ap.broadcast_to((128, 2))
    alpha = sp.tile([128, 2], F32, tag=f"al{tag}")
    v.tensor_tensor(alpha[:], cw[:, base:base + 2], invb, ALU.mult)
    beta = sp.tile([128, 2], F32, tag=f"be{tag}")
    v.tensor_tensor(beta[:], cw[:, base + 2:base + 4], invb, ALU.mult)
    tb = sp.tile([128, 2], F32, tag=f"tb{tag}")
    v.tensor_tensor(tb[:], cw[:, base + 4:base + 6],
                    muinv_ap.broadcast_to((128, 2)), ALU.mult)
    v.tensor_tensor(beta[:], beta[:], tb[:], ALU.add)
    if has_be:
        v.tensor_tensor(beta[:], beta[:], cw[:, base + 6:base + 8], ALU.add)
    return alpha, beta


def build_program(imms, has_be):
    nc = bacc.Bacc("TRN2", target_bir_lowering=False, debug=False, num_devices=8)

    audio_s = nc.dram_tensor("audio_s", [128, Ta * F], F32, kind="ExternalInput")
    video_f = nc.dram_tensor("video_f", [128, REP * Tv], F32, kind="ExternalInput")
    cw_d = nc.dram_tensor("cw", [128, NCW], F32, kind="ExternalInput")
    out_d = nc.dram_tensor("out_c", [128, 2 * Tv], F32, kind="ExternalOutput")

    offs = [0, 1024, 2048, 2560, 3072, 3584]
    sizes = [1024, 1024, 512, 512, 512, 512]
    qb_zero = has_be[4] if len(has_be) > 4 else False
    fast_gate = not has_be[1]
    fast_val = not has_be[0]
    assert fast_gate and fast_val, "v5 assumes p1_b=p2_b=0, p1_be=p2_be=0"

    with tile.TileContext(nc) as tc:
        with (
            tc.tile_pool(name="big", bufs=1) as bigp,
            tc.tile_pool(name="z", bufs=2) as zp,
            tc.tile_pool(name="scr", bufs=2) as scrp,
            tc.tile_pool(name="sp", bufs=1) as sp,
            tc.tile_pool(name="psum", bufs=2, space="PSUM") as psp,
        ):
            v = nc.vector
            g = nc.gpsimd
            A = bigp.tile([128, Ta * F], F32, tag="A")
            vf = bigp.tile([128, REP * Tv], F32, tag="vf")
            cw = bigp.tile([128, NCW], F32, tag="cw")
            ones = bigp.tile([128, 128], F32, tag="ones")
            magic = bigp.tile([128, 2], I32, tag="magic")

            # ---- input DMAs (v1 ordering: small tensors first so the video
            # chain runs inside the audio window; audio split across rings)
            VH = REP * Tv // 2
            nc.sync.dma_start(vf[:, :VH], video_f[:, :VH])
            nc.scalar.dma_start(cw[:], cw_d[:])
            nc.scalar.dma_start(vf[:, VH:], video_f[:, VH:])
            dma_eng = [nc.sync, nc.scalar]
            for c in range(6):
                dma_eng[c % 2].dma_start(A[:, offs[c]:offs[c] + sizes[c]],
                                         audio_s[:, offs[c]:offs[c] + sizes[c]])
            g.memset(ones[:], 1.0)
            g.memset(magic[:], RSQRT_MAGIC)

            # ---- video stats per half
            T2vc = sp.tile([128, 4], F32, tag="T2vc")
            T1vc = sp.tile([128, 4], F32, tag="T1vc")
            for hh in range(2):
                hs = slice(VH * hh, VH * (hh + 1))
                v.reduce_sum(T1vc[:, 2 * hh:2 * hh + 2],
                             vf[:, hs].rearrange("p (r t) -> p r t", t=Tv),
                             axis=mybir.AxisListType.X)
                vsq = scrp.tile([128, VH], F32, tag="vsq")
                nc.scalar.activation(vsq[:], vf[:, hs], AF.Square)
                v.reduce_sum(T2vc[:, 2 * hh:2 * hh + 2],
                             vsq[:].rearrange("p (r t) -> p r t", t=Tv),
                             axis=mybir.AxisListType.X)
            pt1 = sp.tile([128, 16], F32, tag="pt1")
            v.tensor_tensor(pt1[:].rearrange("p (g r) -> p g r", r=4),
                            T1vc[:].unsqueeze(1).broadcast_to((128, 4, 4)),
                            cw[:, C_VT1:C_VT1 + 16].rearrange(
                                "p (g r) -> p g r", r=4), ALU.mult)
            pv1 = sp.tile([128, 4], F32, tag="pv1")
            v.reduce_sum(pv1[:], pt1[:].rearrange("p (g r) -> p g r", r=4),
                         axis=mybir.AxisListType.X)
            pt2 = sp.tile([128, 8], F32, tag="pt2")
            v.tensor_tensor(pt2[:].rearrange("p (g r) -> p g r", r=4),
                            T2vc[:].unsqueeze(1).broadcast_to((128, 2, 4)),
                            cw[:, C_VT2:C_VT2 + 8].rearrange(
                                "p (g r) -> p g r", r=4), ALU.mult)
            pv2 = sp.tile([128, 2], F32, tag="pv2")
            v.reduce_sum(pv2[:], pt2[:].rearrange("p (g r) -> p g r", r=4),
                         axis=mybir.AxisListType.X)
            ps_v1 = psp.tile([128, 4], F32, tag="ps_v1")
            nc.tensor.matmul(ps_v1[:], ones[:], pv1[:])
            ps_v2 = psp.tile([128, 2], F32, tag="ps_v2")
            nc.tensor.matmul(ps_v2[:], ones[:], pv2[:])
            inv34, muinv34 = _derive_invs(nc, sp, magic, ps_v1[:, 0:2],
                                          ps_v2[:, 0:2], ps_v1[:, 2:4],
                                          imms[1], "v")
            A3p, B3p = _coef_pair(nc, sp, cw, C_W3GM, inv34[:, 0:1],
                                  muinv34[:, 0:1], has_be[2], "s", v=g)
            A4p, B4p = _coef_pair(nc, sp, cw, C_W4G4, inv34[:, 1:2],
                                  muinv34[:, 1:2], has_be[3], "k", v=g)
            VBOUND = 12.0
            aA3 = sp.tile([128, 2], F32, tag="aA3")
            v.tensor_scalar(aA3[:, 0:1], A3p[:, 0:1], -1.0, A3p[:, 0:1],
                            ALU.mult, ALU.max)
            v.tensor_scalar(aA3[:, 1:2], A3p[:, 1:2], -1.0, A3p[:, 1:2],
                            ALU.mult, ALU.max)
            bEp = sp.tile([128, 2], F32, tag="bEp")
            v.tensor_scalar(bEp[:], aA3[:], -VBOUND, None, ALU.mult)

            # ---- audio scans per chunk
            SA = sp.tile([128, Ta], F32, tag="SA")
            T2c = sp.tile([128, 6], F32, tag="T2c")
            for c in range(6):
                v.reduce_sum(SA[:, offs[c] // F:(offs[c] + sizes[c]) // F],
                             A[:, offs[c]:offs[c] + sizes[c]].rearrange(
                                 "p (t f) -> p t f", f=F),
                             axis=mybir.AxisListType.X)
                sq = scrp.tile([128, 1024], F32, tag="sq")
                nc.scalar.activation(sq[:, :sizes[c]],
                                     A[:, offs[c]:offs[c] + sizes[c]], AF.Square,
                                     accum_out=T2c[:, c:c + 1])

            # ---- merged mu + variance partition reduce
            T1 = sp.tile([128, 1], F32, tag="T1")
            v.reduce_sum(T1[:], SA[:], axis=mybir.AxisListType.X)
            T2 = sp.tile([128, 1], F32, tag="T2")
            v.reduce_sum(T2[:], T2c[:], axis=mybir.AxisListType.X)
            nq = 4 if qb_zero else 6
            P4 = sp.tile([128, nq], F32, tag="P4")
            v.tensor_tensor(P4[:, 0:2], T1[:].broadcast_to((128, 2)),
                            cw[:, C_W1S:C_W1S + 2], ALU.mult)
            v.tensor_tensor(P4[:, 2:4], T2[:].broadcast_to((128, 2)),
                            cw[:, C_W1SQ:C_W1SQ + 2], ALU.mult)
            if not qb_zero:
                v.tensor_tensor(P4[:, 4:6], T1[:].broadcast_to((128, 2)),
                                cw[:, C_WB1:C_WB1 + 2], ALU.mult)
            ps_a = psp.tile([128, nq], F32, tag="ps_a")
            nc.tensor.matmul(ps_a[:], ones[:], P4[:])
            invN1, mu1_add, q1_add, _, mu2_add, q2_add = imms[0]
            # relu bias be2r_i straight from PSUM on ACT (host-folded cols)
            be2r = sp.tile([128, 2], F32, tag="be2r")
            for i in range(2):
                nc.scalar.activation(be2r[:, i:i + 1], ps_a[:, 1:2], AF.Identity,
                                     bias=cw[:, C_BERB + i:C_BERB + i + 1],
                                     scale=cw[:, C_BERS + i:C_BERS + i + 1])
            mu12 = sp.tile([128, 2], F32, tag="mu12")
            v.tensor_scalar(mu12[:, 0:1], ps_a[:, 0:1], invN1, mu1_add,
                            ALU.mult, ALU.add)
            v.tensor_scalar(mu12[:, 1:2], ps_a[:, 1:2], invN1, mu2_add,
                            ALU.mult, ALU.add)
            qb = None if qb_zero else ps_a[:, 4:6]
            inv12, muinv12 = _derive_invs(nc, sp, magic, mu12[:], ps_a[:, 2:4],
                                          qb, imms[0], "a", mu_ready=True)

            # gate fold: A4pp/B4pp = A4p/B4p * inv2 (both residues)
            A4pp = sp.tile([128, 2], F32, tag="A4pp")
            g.tensor_tensor(A4pp[:], A4p[:],
                            inv12[:, 1:2].broadcast_to((128, 2)), ALU.mult)
            B4pp = sp.tile([128, 2], F32, tag="B4pp")
            g.tensor_tensor(B4pp[:], B4p[:],
                            inv12[:, 1:2].broadcast_to((128, 2)), ALU.mult)
            # W_i = A4pp_i * v + B4pp_i  (GpSimd TT pairs, in parallel with relus)
            Ws = []
            for i in range(2):
                W = scrp.tile([128, Tv], F32, tag=f"W{i}")
                g.tensor_tensor(W[:], vf[:, Tv * i:Tv * (i + 1)],
                                A4pp[:, i:i + 1].broadcast_to((128, Tv)),
                                ALU.mult)
                g.tensor_tensor(W[:], W[:],
                                B4pp[:, i:i + 1].broadcast_to((128, Tv)),
                                ALU.add)
                Ws.append(W)
            # ---- gate relus (ACT, bf16 z) + DVE tree sums.  The EXPs slot
            # into the ACT stream between the two residue groups (v1 trick):
            # their deps (video chain) are ready by then and the softmax
            # denominators land well before the fusion needs them.
            Es, ses = [], []
            SG = sp.tile([128, 2 * Ta], F32, tag="SG")
            RQ = 2048
            with nc.allow_low_precision(reason="gate sums tolerate bf16"):
                for i in range(2):
                    if i == 1:
                        for j in range(2):
                            E = scrp.tile([128, Tv], F32, tag=f"E{j}")
                            se = sp.tile([128, 1], F32, tag=f"se{j}")
                            nc.scalar.activation(E[:], vf[:, Tv * j:Tv * (j + 1)],
                                                 AF.Exp, bias=bEp[:, j:j + 1],
                                                 scale=A3p[:, j:j + 1],
                                                 accum_out=se[:])
                            Es.append(E)
                            ses.append(se)
                        rc0 = sp.tile([128, 1], F32, tag="rc0")
                        v.reciprocal(rc0[:], ses[0][:])
                        rc1 = sp.tile([128, 1], F32, tag="rc1")
                        v.reciprocal(rc1[:], ses[1][:])
                    for cc in range(2):
                        z1 = zp.tile([128, RQ], BF16, tag=f"z{i}{cc}")
                        nc.scalar.activation(
                            z1[:], A[:, RQ * cc:RQ * (cc + 1)], AF.Relu,
                            bias=be2r[:, i:i + 1],
                            scale=cw[:, C_W2G2 + i:C_W2G2 + i + 1])
                        t1a = zp.tile([128, 32 * 32], BF16, tag=f"ta{i}{cc}")
                        z13 = z1[:].rearrange("p (t f) -> p t f", f=F)
                        v.tensor_tensor(t1a[:].rearrange("p (t f) -> p t f", f=32),
                                        z13[:, :, 0:32], z13[:, :, 32:64], ALU.add)
                        t1b = zp.tile([128, 32 * 16], BF16, tag=f"tb{i}{cc}")
                        ta13 = t1a[:].rearrange("p (t f) -> p t f", f=32)
                        v.tensor_tensor(t1b[:].rearrange("p (t f) -> p t f", f=16),
                                        ta13[:, :, 0:16], ta13[:, :, 16:32], ALU.add)
                        t1c = zp.tile([128, 32 * 8], BF16, tag=f"tc{i}{cc}")
                        tb13 = t1b[:].rearrange("p (t f) -> p t f", f=16)
                        v.tensor_tensor(t1c[:].rearrange("p (t f) -> p t f", f=8),
                                        tb13[:, :, 0:8], tb13[:, :, 8:16], ALU.add)
                        v.reduce_sum(SG[:, Ta * i + 32 * cc:Ta * i + 32 * (cc + 1)],
                                     t1c[:].rearrange("p (t f) -> p t f", f=8),
                                     axis=mybir.AxisListType.X)

            # val-path folded coefficients: SVp_i = sc_i*SA + bi_i
            be1r = sp.tile([128, 2], F32, tag="be1r")
            g.tensor_tensor(be1r[:], cw[:, C_G1:C_G1 + 2],
                            mu12[:, 0:1].broadcast_to((128, 2)), ALU.mult)
            g.tensor_tensor(be1r[:], be1r[:], cw[:, C_BG1:C_BG1 + 2], ALU.add)
            rcp = sp.tile([128, 2], F32, tag="rcp")
            g.tensor_tensor(rcp[:, 0:1], rc0[:], inv12[:, 0:1], ALU.mult)
            g.tensor_tensor(rcp[:, 1:2], rc1[:], inv12[:, 0:1], ALU.mult)
            svs = sp.tile([128, 2], F32, tag="svs")
            g.tensor_tensor(svs[:], rcp[:], cw[:, C_W1G1:C_W1G1 + 2], ALU.mult)
            svb = sp.tile([128, 2], F32, tag="svb")
            g.tensor_tensor(svb[:], rcp[:], be1r[:], ALU.mult)
            g.tensor_scalar(svb[:], svb[:], float(F), None, ALU.mult)

            # ---- fusion: ot_i = P_i + SG_i (x4) * W_i
            out_sb = sp.tile([128, 2 * Tv], F32, tag="out_sb")
            for i in range(2):
                vblk = vf[:, Tv * i:Tv * (i + 1)]
                SVp = sp.tile([128, Ta], F32, tag=f"SVp{i}")
                nc.scalar.activation(SVp[:], SA[:], AF.Identity,
                                     bias=svb[:, i:i + 1],
                                     scale=svs[:, i:i + 1])
                P = scrp.tile([128, Tv], F32, tag=f"P{i}")
                g.tensor_tensor(P[:].rearrange("p (t k) -> p t k", k=4),
                                Es[i][:].rearrange("p (t k) -> p t k", k=4),
                                SVp[:].unsqueeze(2).broadcast_to((128, Ta, 4)),
                                ALU.mult)
                g.tensor_tensor(P[:], P[:], vblk, ALU.add)
                sg_blk = SG[:, Ta * i:Ta * (i + 1)]
                eng = g if i == 0 else v
                q = scrp.tile([128, Tv], F32, tag=f"q{i}")
                eng.tensor_tensor(q[:].rearrange("p (t k) -> p t k", k=4),
                                  Ws[i][:].rearrange("p (t k) -> p t k", k=4),
                                  sg_blk.unsqueeze(2).broadcast_to((128, Ta, 4)),
                                  ALU.mult)
                eng.tensor_tensor(out_sb[:, Tv * i:Tv * (i + 1)], q[:], P[:],
                                  ALU.add)
            nc.sync.dma_start(out_d[:], out_sb[:])
    nc.compile()
    return nc


def _prep_consts(params):
    (p1_w, p1_b, p1_g, p1_be, p2_w, p2_b, p2_g, p2_be,
     f1_w, f1_b, f1_g, f1_be, f2_w, f2_b, f2_g, f2_be) = [
        np.asarray(params[k], dtype=np.float64) for k in (
            "p1_w", "p1_b", "p1_g", "p1_be", "p2_w", "p2_b", "p2_g", "p2_be",
            "f1_w", "f1_b", "f1_g", "f1_be", "f2_w", "f2_b", "f2_g", "f2_be")]

    def gsum(x, g):
        return x.reshape(-1, g).sum(1)

    w1s, w1sq, wb1 = gsum(p1_w, REP), gsum(p1_w ** 2, REP), gsum(2 * p1_w * p1_b, REP)
    w2s, w2sq, wb2 = gsum(p2_w, REP), gsum(p2_w ** 2, REP), gsum(2 * p2_w * p2_b, REP)
    w3s, w3sq, wb3 = gsum(f1_w, NH), gsum(f1_w ** 2, NH), gsum(2 * f1_w * f1_b, NH)

    invN1 = 1.0 / N1
    mu2_add = Ta * F * p2_b.sum() / N1

    cws = []
    for h in range(2):
        cw = np.zeros((128, NCW), np.float64)
        cw[:, C_W1S], cw[:, C_W2S] = w1s, w2s
        cw[:, C_W1SQ], cw[:, C_W2SQ] = w1sq, w2sq
        cw[:, C_WB1], cw[:, C_WB2] = wb1, wb2
        order = [2 * h, 2 * h + 1] + [r for r in range(4) if r not in (2 * h, 2 * h + 1)]
        for pos, r in enumerate(order):
            cv = 4 * np.arange(128) + r
            cw[:, C_VT1 + 0 + pos] = w3s[cv]
            cw[:, C_VT1 + 4 + pos] = f2_w[cv]
            cw[:, C_VT1 + 8 + pos] = wb3[cv]
            cw[:, C_VT1 + 12 + pos] = 2 * f2_w[cv] * f2_b[cv]
            cw[:, C_VT2 + 0 + pos] = w3sq[cv]
            cw[:, C_VT2 + 4 + pos] = f2_w[cv] ** 2
        for i in range(2):
            cv = 4 * np.arange(128) + (2 * h + i)
            cw[:, C_W2G2 + i] = (p2_w * p2_g)[cv]
            cw[:, C_BG2 + i] = (p2_b * p2_g)[cv]
            cw[:, C_G2 + i] = -p2_g[cv]
            cw[:, C_BE2 + i] = p2_be[cv]
            cw[:, C_W1G1 + i] = (p1_w * p1_g)[cv]
            cw[:, C_BG1 + i] = (p1_b * p1_g)[cv]
            cw[:, C_G1 + i] = -p1_g[cv]
            cw[:, C_BE1 + i] = p1_be[cv]
            cw[:, C_W3GM + i] = (f1_w * f1_g).reshape(Cv, NH).mean(1)[cv]
            cw[:, C_BG3M + i] = (f1_b * f1_g).reshape(Cv, NH).mean(1)[cv]
            cw[:, C_G3M + i] = -f1_g.reshape(Cv, NH).mean(1)[cv]
            cw[:, C_BE3M + i] = f1_be.reshape(Cv, NH).mean(1)[cv]
            cw[:, C_W4G4 + i] = (f2_w * f2_g)[cv]
            cw[:, C_BG4 + i] = (f2_b * f2_g)[cv]
            cw[:, C_G4 + i] = -f2_g[cv]
            cw[:, C_BE4 + i] = f2_be[cv]
            # be2r_i = (-g2*invN1)*ps2 + (-g2*mu2_add + b2*g2)
            cw[:, C_BERS + i] = (-p2_g[cv]) * invN1
            cw[:, C_BERB + i] = (-p2_g[cv]) * mu2_add + (p2_b * p2_g)[cv]
        cws.append(cw.astype(np.float32))

    imm_a = (invN1, Ta * F * p1_b.sum() / N1, Ta * F * (p1_b ** 2).sum() / N1 + EPS,
             invN1, mu2_add, Ta * F * (p2_b ** 2).sum() / N1 + EPS)
    imm_v = (1.0 / N3, Tv * f1_b.sum() / N3, Tv * (f1_b ** 2).sum() / N3 + EPS,
             1.0 / N4, Tv * f2_b.sum() / N4, Tv * (f2_b ** 2).sum() / N4 + EPS)
    imms = (tuple(float(x) for x in imm_a), tuple(float(x) for x in imm_v))
    has_be = (bool(np.any(p1_be)), bool(np.any(p2_be)),
              bool(np.any(f1_be)), bool(np.any(f2_be)),
              not (np.any(p1_b) or np.any(p2_b)))
    return cws, imms, has_be


def kernel(**inputs):
    global LAST_EXEC_NS, LAST_RESULTS
    audio = np.ascontiguousarray(np.asarray(inputs["audio"], dtype=np.float32))
    video = np.ascontiguousarray(np.asarray(inputs["video"], dtype=np.float32))
    cws, imms, has_be = _prep_consts(inputs)

    key = ("prog", imms, has_be)
    if key not in _CACHE:
        _CACHE[key] = build_program(imms, has_be)
    nc = _CACHE[key]

    in_maps = []
    for core in range(8):
        b, h = core // 2, core % 2
        vres = video[b].reshape(128, 4, Tv)
        order = [2 * h, 2 * h + 1] + [r for r in range(4) if r not in (2 * h, 2 * h + 1)]
        vfm = np.ascontiguousarray(vres[:, order, :].reshape(128, 4 * Tv))
        in_maps.append({
            "audio_s": np.ascontiguousarray(audio[b].reshape(128, Ta * F)),
            "video_f": vfm,
            "cw": cws[h],
        })

    trace = bool(int(os.environ.get("BASS_KERNEL_TRACE", "0")))
    res = run_bass_kernel_spmd(nc, in_maps, list(range(8)), trace=trace)
    LAST_EXEC_NS = res.exec_time_ns
    LAST_RESULTS = res
    out = np.empty((B, Cv, Tv), np.float32)
    for core in range(8):
        b, h = core // 2, core % 2
        oc = res.results[core]["out_c"].reshape(128, 2, Tv)
        ov = out[b].reshape(128, 4, Tv)
        ov[:, 2 * h, :] = oc[:, 0, :]
        ov[:, 2 * h + 1, :] = oc[:, 1, :]
    return out
